# revision 7
# baseline (speedup 1.0000x reference)
"""Gumbel-Sinkhorn kernel for Trainium2 (8 NeuronCores).

Reference semantics: la0 = logits + gumbel(noise); 20 iterations of
row-logsumexp / col-logsumexp normalization in log domain; out = exp(la).

Because each iteration only ever subtracts a row vector and a column
vector from la, the whole recursion collapses to classic Sinkhorn
scaling on the fixed kernel matrix K = exp(la0):

    v = 1;  repeat 20x: u = 1/(K v);  v = 1/(K^T u)
    out = diag(u) K diag(v)

This needs no per-iteration exp/log over the matrix - only multiply-
accumulate passes, which stay SBUF-resident on each core.

Sharding: rows are split across 8 cores (512 rows each).  Each core
holds its K shard (row-major, [i-part, j-free]) and its transpose
KT ([j-part, i-free]).  Per iteration:
  row pass   r = K v      : per-partition-scalar mult-accumulate over KT
                            tiles + PE ones-matmul partition reduce
  col pass   c_part = K^T u: same over K tiles; 16KB AllReduce combines
                            the 8 partial column sums.
"""

import sys

sys.path.insert(0, "/opt/trn_rl_repo")

import numpy as np
from contextlib import ExitStack

N_CORES = 8
DIM = 4096
ROWS = DIM // N_CORES  # 512 rows per core
RT = ROWS // 128       # 4 row tiles of 128 partitions
CT = DIM // 128        # 32 col tiles of 128 partitions
N_ITERS = 20
EPS = 1e-10

_CACHE = {}


def _build():
    from concourse import bass, tile, bacc
    from concourse import mybir

    fp32 = mybir.dt.float32
    Alu = mybir.AluOpType
    Act = mybir.ActivationFunctionType

    nc = bacc.Bacc(
        "TRN2",
        target_bir_lowering=False,
        debug=False,
        num_devices=N_CORES,
    )

    logits_d = nc.dram_tensor("logits", [ROWS, DIM], fp32, kind="ExternalInput")
    noise_d = nc.dram_tensor("noise", [ROWS, DIM], fp32, kind="ExternalInput")
    out_d = nc.dram_tensor("out", [ROWS, DIM], fp32, kind="ExternalOutput")

    ident_np = np.eye(128, dtype=np.float32)
    ident_d = nc.inline_tensor(ident_np, name="ident128")

    rg = [list(range(N_CORES))]

    with tile.TileContext(nc) as tc, ExitStack() as ctx:
        persist = ctx.enter_context(tc.tile_pool(name="persist", bufs=1))
        # K  : [128, RT*4096]  slice t -> rows 128t..128t+128, all cols
        # KT : [128, CT*512]   slice s -> cols 128s..128s+128, all my rows
        K = persist.tile([128, RT * DIM], fp32, tag="K")
        KT = persist.tile([128, CT * ROWS], fp32, tag="KT")
        ones_col = persist.tile([128, 1], fp32, tag="ones_col")
        eps_col = persist.tile([128, 1], fp32, tag="eps_col")
        ones_row = persist.tile([1, 128], fp32, tag="ones_row")
        ident = persist.tile([128, 128], fp32, tag="ident")
        u_sb = persist.tile([128, RT], fp32, tag="u")
        v_sb = persist.tile([128, CT], fp32, tag="v")
        vrow = persist.tile([1, DIM], fp32, tag="vrow")
        Vb = persist.tile([128, DIM], fp32, tag="Vb")

        nc.vector.memset(ones_col[:], 1.0)
        nc.vector.memset(eps_col[:], EPS)
        nc.vector.memset(ones_row[:], 1.0)
        nc.sync.dma_start(out=ident[:], in_=ident_d.ap())
        nc.vector.memset(v_sb[:], 1.0)

        # ---------------- setup: K = exp(logits - log(-log(noise+eps)+eps))
        HW = 2048  # half-width chunks to bound staging SBUF
        with tc.tile_pool(name="load", bufs=2) as ld:
            for t in range(RT):
                for h in range(DIM // HW):
                    sl = slice(h * HW, (h + 1) * HW)
                    lg = ld.tile([128, HW], fp32, tag="lg")
                    nz = ld.tile([128, HW], fp32, tag="nz")
                    nc.sync.dma_start(
                        out=lg[:], in_=logits_d[t * 128 : (t + 1) * 128, sl]
                    )
                    nc.sync.dma_start(
                        out=nz[:], in_=noise_d[t * 128 : (t + 1) * 128, sl]
                    )
                    # a = ln(noise + eps)
                    nc.scalar.activation(nz[:], nz[:], Act.Ln, bias=eps_col[:], scale=1.0)
                    # m = ln(-a + eps)
                    nc.scalar.activation(nz[:], nz[:], Act.Ln, bias=eps_col[:], scale=-1.0)
                    # s = logits - m
                    nc.vector.tensor_tensor(
                        out=lg[:], in0=lg[:], in1=nz[:], op=Alu.subtract
                    )
                    # K tile chunk
                    nc.scalar.activation(
                        K[:, t * DIM + h * HW : t * DIM + (h + 1) * HW],
                        lg[:],
                        Act.Exp,
                    )

        # ---------------- build KT via PE transposes
        with tc.tile_pool(name="tp", bufs=4, space="PSUM") as tp:
            for s in range(CT):
                pt = tp.tile([128, 512], fp32, tag="pt")
                for t in range(RT):
                    nc.tensor.transpose(
                        pt[:, t * 128 : (t + 1) * 128],
                        K[:, t * DIM + s * 128 : t * DIM + (s + 1) * 128],
                        ident[:],
                    )
                dst = KT[:, s * ROWS : (s + 1) * ROWS]
                if s % 2 == 0:
                    nc.vector.tensor_copy(dst, pt[:])
                else:
                    nc.scalar.copy(dst, pt[:])

        # ---------------- Sinkhorn iterations
        with (
            tc.tile_pool(name="acc", bufs=2) as accp,
            tc.tile_pool(name="rp", bufs=2, space="PSUM") as rp,
            tc.tile_pool(name="cc", bufs=2, space="DRAM") as ccp,
            tc.tile_pool(name="sm", bufs=3) as smp,
        ):
            v_cur = v_sb
            for it in range(N_ITERS):
                # --- row pass: accR[p, i] = sum_s KT[s][p, i] * v[128s+p]
                accR = accp.tile([128, ROWS], fp32, tag="accR")
                nc.vector.tensor_scalar(
                    accR[:], KT[:, 0:ROWS], v_cur[:, 0:1], None, Alu.mult
                )
                for s in range(1, CT):
                    nc.vector.scalar_tensor_tensor(
                        out=accR[:],
                        in0=KT[:, s * ROWS : (s + 1) * ROWS],
                        scalar=v_cur[:, s : s + 1],
                        in1=accR[:],
                        op0=Alu.mult,
                        op1=Alu.add,
                    )
                # partition-reduce -> r [128, RT]
                pr = rp.tile([128, RT], fp32, tag="pr")
                for t in range(RT):
                    nc.tensor.matmul(
                        pr[:, t : t + 1],
                        lhsT=accR[:, t * 128 : (t + 1) * 128],
                        rhs=ones_col[:],
                        start=True,
                        stop=True,
                    )
                nc.vector.reciprocal(u_sb[:], pr[:])

                # --- col pass: accC[p, j] = sum_t K[t][p, j] * u[128t+p]
                accC = accp.tile([128, DIM], fp32, tag="accC")
                nc.vector.tensor_scalar(
                    accC[:], K[:, 0:DIM], u_sb[:, 0:1], None, Alu.mult
                )
                for t in range(1, RT):
                    nc.vector.scalar_tensor_tensor(
                        out=accC[:],
                        in0=K[:, t * DIM : (t + 1) * DIM],
                        scalar=u_sb[:, t : t + 1],
                        in1=accC[:],
                        op0=Alu.mult,
                        op1=Alu.add,
                    )
                # partition-reduce -> c partials [128, CT]
                pc = rp.tile([128, CT], fp32, tag="pc")
                for s in range(CT):
                    nc.tensor.matmul(
                        pc[:, s : s + 1],
                        lhsT=accC[:, s * 128 : (s + 1) * 128],
                        rhs=ones_col[:],
                        start=True,
                        stop=True,
                    )
                cp = smp.tile([128, CT], fp32, tag="cp")
                nc.vector.tensor_copy(cp[:], pc[:])

                # --- all-reduce column sums across the 8 cores
                cc_in = ccp.tile([128, CT], fp32, tag="cc_in")
                cc_out = ccp.tile([128, CT], fp32, tag="cc_out")
                nc.sync.dma_start(out=cc_in[:], in_=cp[:])
                nc.gpsimd.collective_compute(
                    "AllReduce",
                    Alu.add,
                    replica_groups=rg,
                    ins=[cc_in.opt()],
                    outs=[cc_out.opt()],
                )
                csb = smp.tile([128, CT], fp32, tag="csb")
                nc.sync.dma_start(out=csb[:], in_=cc_out[:])
                vn = smp.tile([128, CT], fp32, tag="vn")
                nc.vector.reciprocal(vn[:], csb[:])
                v_cur = vn

        # ---------------- output: out = u * K * v
        with (
            tc.tile_pool(name="ob", bufs=2) as ob,
            tc.tile_pool(name="op", bufs=4, space="PSUM") as op,
        ):
            # v row vector [1, DIM] via PE transposes of v columns
            for b in range(8):
                pv = op.tile([1, 512], fp32, tag="pv")
                for q in range(4):
                    s = b * 4 + q
                    nc.tensor.transpose(
                        pv[:, q * 128 : (q + 1) * 128],
                        v_cur[:, s : s + 1],
                        ident[:],
                    )
                nc.vector.tensor_copy(vrow[:, b * 512 : (b + 1) * 512], pv[:])
            # broadcast to all partitions: Vb = ones_row^T @ vrow
            for b in range(8):
                pb = op.tile([128, 512], fp32, tag="pb")
                nc.tensor.matmul(
                    pb[:],
                    lhsT=ones_row[:],
                    rhs=vrow[:, b * 512 : (b + 1) * 512],
                    start=True,
                    stop=True,
                )
                dst = Vb[:, b * 512 : (b + 1) * 512]
                if b % 2 == 0:
                    nc.vector.tensor_copy(dst, pb[:])
                else:
                    nc.scalar.copy(dst, pb[:])
            for t in range(RT):
                o = ob.tile([128, DIM], fp32, tag="o")
                nc.vector.scalar_tensor_tensor(
                    out=o[:],
                    in0=K[:, t * DIM : (t + 1) * DIM],
                    scalar=u_sb[:, t : t + 1],
                    in1=Vb[:],
                    op0=Alu.mult,
                    op1=Alu.mult,
                )
                nc.sync.dma_start(
                    out=out_d[t * 128 : (t + 1) * 128, :], in_=o[:]
                )

    nc.compile()
    return nc


def _get_nc():
    if "nc" not in _CACHE:
        _CACHE["nc"] = _build()
    return _CACHE["nc"]


def kernel(logits: np.ndarray, noise: np.ndarray) -> np.ndarray:
    from concourse import bass_utils

    nc = _get_nc()
    logits = np.ascontiguousarray(np.asarray(logits, dtype=np.float32))
    noise = np.ascontiguousarray(np.asarray(noise, dtype=np.float32))
    in_maps = [
        {
            "logits": logits[c * ROWS : (c + 1) * ROWS],
            "noise": noise[c * ROWS : (c + 1) * ROWS],
        }
        for c in range(N_CORES)
    ]
    res = bass_utils.run_bass_kernel_spmd(nc, in_maps, core_ids=list(range(N_CORES)))
    shards = [res.results[c]["out"] for c in range(N_CORES)]
    return np.concatenate(shards, axis=0).astype(np.float32)


if __name__ == "__main__":
    rng = np.random.default_rng(0)
    lg = rng.standard_normal((DIM, DIM), dtype=np.float32)
    nz = rng.random((DIM, DIM), dtype=np.float32)
    out = kernel(lg, nz)
    print(out.shape, out.dtype, np.isfinite(out).all())
